# revision 8
# baseline (speedup 1.0000x reference)
"""Distributed trace-polynomial Bass kernel for trn2 (8 NeuronCores).

Problem: x [65536,16,16] f32, coef [10,4].
  t_u(b) = trace(x_b^(u+2)), u=0..9
  out[b] = sum_uj coef[u,j] * t_u^(j+1) / 256^(u+j+1)

Device math (per core, 8192 samples, all sample data bf16):
  z := x^T (per-sample transpose, prepared host-side, shipped as input)
  chain: z^(a+1) = z . z^a   (TE matmul, stationary = x expanded to an
         8-sample block-diagonal [128,128]; out = lhsT^T @ rhs = z . rhs)
  traces: tr(x^(a+1)) = <z^a, x>_F per sample
         = elementwise product (DVE) -> partition fold over i (TE with a
           constant selector stationary) -> 16-col segment fold (DVE
           tensor_reduce axis=X).
  features: S = t/256; out = sum_uj coef[u,j] 256^-u S^(j+1)  (DVE + TE fold)

Sharding: pure data parallel, batch split 8 ways across cores; coef only
enters via a tiny host-computed weight matrix; output gathered to [65536].
bf16 storage keeps rel-l2 ~2e-3 vs f64 reference (tolerance 2e-2).
"""

import numpy as np
from contextlib import ExitStack

import ml_dtypes

from concourse import bass, bacc, tile, mybir
from concourse.bass_utils import run_bass_kernel_spmd

B, N = 65536, 16
ROWS, COLS = 10, 4
M = 8           # cores
BS = B // M     # 8192 samples per core
CH = 16         # chunks per core
CS = BS // CH   # 512 samples per chunk
G = CS // 8     # 64 groups (of 8 samples) per chunk
NSTEP = 10      # pairs a=1..10 -> traces t_2..t_11 (9 chain matmuls)

BF16 = mybir.dt.bfloat16
F32 = mybir.dt.float32

_cached = None


def _consts(coef: np.ndarray):
    # row layout of the 96-partition trace tile: r = 32*(u//4) + 8*(u%4) + s
    wmat = np.zeros((96, COLS), np.float32)
    self8 = np.zeros((96, 8), np.float32)
    for u in range(ROWS):
        base = 32 * (u // 4) + 8 * (u % 4)
        for s in range(8):
            wmat[base + s, :] = coef[u, :] * (256.0 ** (-u))
            self8[base + s, s] = 1.0
    # sel4[:, 32q:32q+32]: lhsT mapping partition (s,i) -> within-slice col 8q+s
    sel4 = np.zeros((128, 128), np.float32)
    for q in range(4):
        for s in range(8):
            for i in range(16):
                sel4[16 * s + i, 32 * q + 8 * q + s] = 1.0
    return (
        sel4.astype(ml_dtypes.bfloat16),
        self8.astype(np.float32),
        wmat,
    )


def build():
    nc = bacc.Bacc("TRN2", target_bir_lowering=False, debug=False, num_devices=M)

    x_d = nc.dram_tensor("x", [BS, 256], BF16, kind="ExternalInput").ap()
    z_d = nc.dram_tensor("z", [BS, 256], BF16, kind="ExternalInput").ap()
    sel4_d = nc.dram_tensor("sel4", [128, 128], BF16, kind="ExternalInput").ap()
    self8_d = nc.dram_tensor("self8", [96, 8], F32, kind="ExternalInput").ap()
    wmat_d = nc.dram_tensor("wmat", [96, COLS], F32, kind="ExternalInput").ap()
    out_d = nc.dram_tensor("out", [BS], F32, kind="ExternalOutput").ap()

    with tile.TileContext(nc) as tc:
        with ExitStack() as ctx:
            consts = ctx.enter_context(tc.tile_pool(name="consts", bufs=1))
            xexp_p = ctx.enter_context(tc.tile_pool(name="xexp", bufs=1))
            data = ctx.enter_context(tc.tile_pool(name="data", bufs=3))
            pows = ctx.enter_context(tc.tile_pool(name="pows", bufs=3))
            psum2 = ctx.enter_context(tc.tile_pool(name="psum2", bufs=3, space="PSUM"))
            psum1 = ctx.enter_context(tc.tile_pool(name="psum1", bufs=1, space="PSUM"))
            trc = ctx.enter_context(tc.tile_pool(name="trace", bufs=1))
            fin = ctx.enter_context(tc.tile_pool(name="fin", bufs=1))

            sel4_t = consts.tile([128, 128], BF16)
            nc.sync.dma_start(sel4_t[:], sel4_d[:])
            self8_t = consts.tile([96, 8], F32)
            nc.sync.dma_start(self8_t[:], self8_d[:])
            wmat_t = consts.tile([96, COLS], F32)
            nc.sync.dma_start(wmat_t[:], wmat_d[:])

            # trace accumulator [96, CH*G] f32, filled per chunk
            T_t = trc.tile([96, CH * G], F32)

            # expanded block-diag x stationaries, manually double buffered;
            # off-diagonal zeros are written once and persist
            xexp_ts = [
                xexp_p.tile([128, G * 128], BF16, tag=f"xe{i}", name=f"xexp{i}")
                for i in range(2)
            ]
            nc.vector.memset(xexp_ts[0][:], 0.0)
            nc.vector.memset(xexp_ts[1][:], 0.0)

            for ch in range(CH):
                xexp_t = xexp_ts[ch % 2]
                xs = x_d[CS * ch : CS * (ch + 1), :]
                zs = z_d[CS * ch : CS * (ch + 1), :]

                # load x expanded: slot s -> partitions 16s..16s+16, free 128c+16s+j
                xr = xs.rearrange("(c s) (i j) -> s i c j", s=8, i=16)
                xer = xexp_t.rearrange("p (c w) -> p c w", w=128)
                for s in range(8):
                    nc.sync.dma_start(
                        xer[16 * s : 16 * s + 16, :, 16 * s : 16 * s + 16], xr[s]
                    )

                # compact tiles [128, 16G]: partition (s,i), free (c,j)
                xc_t = data.tile([128, 16 * G], BF16, tag="xc")
                nc.sync.dma_start(
                    xc_t.rearrange("p (c j) -> p c j", j=16),
                    xs.rearrange("(c s) (i j) -> (s i) c j", s=8, i=16),
                )
                zc_t = data.tile([128, 16 * G], BF16, tag="zc")
                nc.sync.dma_start(
                    zc_t.rearrange("p (c j) -> p c j", j=16),
                    zs.rearrange("(c s) (i j) -> (s i) c j", s=8, i=16),
                )

                # per-chunk trace psum [96, 16G] f32; strips accumulate 4 powers
                ps_tr = psum1.tile([96, 16 * G], F32, tag="pstr")

                zprev = zc_t  # z^1
                for a in range(1, NSTEP + 1):
                    u = a - 1
                    strip, q = u // 4, u % 4
                    # trace pair product P = z^a (.) x
                    P_t = pows.tile([128, 16 * G], BF16, tag="pair")
                    nc.vector.tensor_tensor(
                        P_t[:], zprev[:], xc_t[:], mybir.AluOpType.mult
                    )
                    # selector fold over i into partition strip rows 8q+s
                    last_q = 3 if strip < 2 else 1
                    for h in range(0, 16 * G, 512):
                        w = min(512, 16 * G - h)
                        nc.tensor.matmul(
                            ps_tr[32 * strip : 32 * strip + 32, h : h + w],
                            sel4_t[:, 32 * q : 32 * q + 32],
                            P_t[:, h : h + w],
                            start=(q == 0),
                            stop=(q == last_q),
                            tile_position=(0, 32 * strip),
                        )

                    if a < NSTEP:
                        # chain step: z^(a+1) = z . z^a per sample
                        ps_z = psum2.tile([128, 16 * G], F32, tag="psz")
                        for c in range(G):
                            nc.tensor.matmul(
                                ps_z[:, 16 * c : 16 * c + 16],
                                xexp_t[:, 128 * c : 128 * c + 128],
                                zprev[:, 16 * c : 16 * c + 16],
                                start=True,
                                stop=True,
                            )
                        znext = pows.tile([128, 16 * G], BF16, tag=f"zn{a % 2}")
                        nc.scalar.copy(znext[:], ps_z[:])
                        zprev = znext

                # fold 16-col segments: [96, (c,16)] -> [96, G]
                nc.vector.tensor_reduce(
                    T_t[:, G * ch : G * (ch + 1)],
                    ps_tr.rearrange("p (c j) -> p c j", j=16),
                    mybir.AxisListType.X,
                    mybir.AluOpType.add,
                )

            # features: S = T/256, G_acc = sum_j W[:,j] * S^(j+1)
            S_t = fin.tile([96, CH * G], F32, tag="S")
            nc.vector.tensor_scalar_mul(S_t[:], T_t[:], 1.0 / 256.0)
            S2_t = fin.tile([96, CH * G], F32, tag="S2")
            nc.vector.tensor_tensor(S2_t[:], S_t[:], S_t[:], mybir.AluOpType.mult)
            S3_t = fin.tile([96, CH * G], F32, tag="S3")
            nc.vector.tensor_tensor(S3_t[:], S2_t[:], S_t[:], mybir.AluOpType.mult)
            S4_t = fin.tile([96, CH * G], F32, tag="S4")
            nc.vector.tensor_tensor(S4_t[:], S2_t[:], S2_t[:], mybir.AluOpType.mult)

            G1_t = fin.tile([96, CH * G], F32, tag="G1")
            nc.vector.tensor_scalar(
                G1_t[:], S_t[:], wmat_t[:, 0:1], None, mybir.AluOpType.mult
            )
            G2_t = fin.tile([96, CH * G], F32, tag="G2")
            nc.vector.scalar_tensor_tensor(
                G2_t[:], S2_t[:], wmat_t[:, 1:2], G1_t[:],
                mybir.AluOpType.mult, mybir.AluOpType.add,
            )
            G3_t = fin.tile([96, CH * G], F32, tag="G3")
            nc.vector.scalar_tensor_tensor(
                G3_t[:], S3_t[:], wmat_t[:, 2:3], G2_t[:],
                mybir.AluOpType.mult, mybir.AluOpType.add,
            )
            G4_t = fin.tile([96, CH * G], F32, tag="G4")
            nc.vector.scalar_tensor_tensor(
                G4_t[:], S4_t[:], wmat_t[:, 3:4], G3_t[:],
                mybir.AluOpType.mult, mybir.AluOpType.add,
            )

            # fold the 10 power-rows into 8 sample rows: out[s, (ch,c)]
            ps_out = psum1.tile([8, CH * G], F32, tag="pstr", name="ps_out")
            for h in range(0, CH * G, 512):
                w = min(512, CH * G - h)
                nc.tensor.matmul(
                    ps_out[:, h : h + w],
                    self8_t[:],
                    G4_t[:, h : h + w],
                    start=True,
                    stop=True,
                )
            out_sb = fin.tile([8, CH * G], F32, tag="outsb")
            nc.vector.tensor_copy(out_sb[:], ps_out[:])
            # out[b], b = 512*ch + 8*c + s; psum cols are (ch,c)
            nc.sync.dma_start(
                out_d.rearrange("(ch c s) -> s (ch c)", ch=CH, c=G, s=8),
                out_sb[:],
            )

    nc.compile()
    return nc


def _prep_inputs(x: np.ndarray, coef: np.ndarray):
    x = np.ascontiguousarray(x, dtype=np.float32).reshape(B, N, N)
    xb = x.astype(ml_dtypes.bfloat16)
    zb = np.ascontiguousarray(xb.transpose(0, 2, 1))
    xb = xb.reshape(M, BS, 256)
    zb = zb.reshape(M, BS, 256)
    sel4, self8, wmat = _consts(np.asarray(coef, dtype=np.float32))
    return [
        {"x": xb[i], "z": zb[i], "sel4": sel4, "self8": self8, "wmat": wmat}
        for i in range(M)
    ]


def kernel(x: np.ndarray, coef: np.ndarray) -> np.ndarray:
    global _cached
    if _cached is None:
        _cached = build()
    in_maps = _prep_inputs(x, coef)
    res = run_bass_kernel_spmd(_cached, in_maps, core_ids=list(range(M)))
    out = np.concatenate(
        [np.asarray(res.results[i]["out"]).reshape(BS) for i in range(M)]
    )
    return out.astype(np.float32)


if __name__ == "__main__":
    rng = np.random.default_rng(0)
    x = (rng.standard_normal((B, N, N)) * 0.5).astype(np.float32)
    coef = (rng.standard_normal((ROWS, COLS)) * np.sqrt(0.5)).astype(np.float32)
    got = kernel(x, coef)
    print(got[:8])


# revision 9
# speedup vs baseline: 1.0012x; 1.0012x over previous
"""Distributed trace-polynomial Bass kernel for trn2 (8 NeuronCores).

Problem: x [65536,16,16] f32, coef [10,4].
  t_u(b) = trace(x_b^(u+2)), u=0..9
  out[b] = sum_uj coef[u,j] * t_u^(j+1) / 256^(u+j+1)

Device math (per core, 8192 samples, all sample data bf16):
  z := x^T (per-sample transpose, prepared host-side, shipped as input)
  chain: z^(a+1) = z . z^a   (TE matmul, stationary = x expanded to an
         8-sample block-diagonal [128,128]; out = lhsT^T @ rhs = z . rhs)
  traces: tr(x^(a+1)) = <z^a, x>_F per sample
         = elementwise product (DVE) -> partition fold over i (TE with a
           constant selector stationary) -> 16-col segment fold (DVE
           tensor_reduce axis=X).
  features: S = t/256; out = sum_uj coef[u,j] 256^-u S^(j+1)  (DVE + TE fold)

Sharding: pure data parallel, batch split 8 ways across cores; coef only
enters via a tiny host-computed weight matrix; output gathered to [65536].
bf16 storage keeps rel-l2 ~2e-3 vs f64 reference (tolerance 2e-2).
"""

import numpy as np
from contextlib import ExitStack

import ml_dtypes

from concourse import bass, bacc, tile, mybir
from concourse.bass_utils import run_bass_kernel_spmd

B, N = 65536, 16
ROWS, COLS = 10, 4
M = 8           # cores
BS = B // M     # 8192 samples per core
CH = 16         # chunks per core
CS = BS // CH   # 512 samples per chunk
G = CS // 8     # 64 groups (of 8 samples) per chunk
NSTEP = 10      # pairs a=1..10 -> traces t_2..t_11 (9 chain matmuls)

BF16 = mybir.dt.bfloat16
F32 = mybir.dt.float32

_cached = None


def _consts(coef: np.ndarray):
    # row layout of the 96-partition trace tile: r = 32*(u//4) + 8*(u%4) + s
    wmat = np.zeros((96, COLS), np.float32)
    self8 = np.zeros((96, 8), np.float32)
    for u in range(ROWS):
        base = 32 * (u // 4) + 8 * (u % 4)
        for s in range(8):
            wmat[base + s, :] = coef[u, :] * (256.0 ** (-u))
            self8[base + s, s] = 1.0
    # sel4[:, 32q:32q+32]: lhsT mapping partition (s,i) -> within-slice col 8q+s
    sel4 = np.zeros((128, 128), np.float32)
    for q in range(4):
        for s in range(8):
            for i in range(16):
                sel4[16 * s + i, 32 * q + 8 * q + s] = 1.0
    return (
        sel4.astype(ml_dtypes.bfloat16),
        self8.astype(np.float32),
        wmat,
    )


def build():
    nc = bacc.Bacc("TRN2", target_bir_lowering=False, debug=False, num_devices=M)

    x_d = nc.dram_tensor("x", [BS, 256], BF16, kind="ExternalInput").ap()
    z_d = nc.dram_tensor("z", [BS, 256], BF16, kind="ExternalInput").ap()
    sel4_d = nc.dram_tensor("sel4", [128, 128], BF16, kind="ExternalInput").ap()
    self8_d = nc.dram_tensor("self8", [96, 8], F32, kind="ExternalInput").ap()
    wmat_d = nc.dram_tensor("wmat", [96, COLS], F32, kind="ExternalInput").ap()
    out_d = nc.dram_tensor("out", [BS], F32, kind="ExternalOutput").ap()

    with tile.TileContext(nc) as tc:
        with ExitStack() as ctx:
            consts = ctx.enter_context(tc.tile_pool(name="consts", bufs=1))
            xexp_p = ctx.enter_context(tc.tile_pool(name="xexp", bufs=1))
            data = ctx.enter_context(tc.tile_pool(name="data", bufs=3))
            pows = ctx.enter_context(tc.tile_pool(name="pows", bufs=3))
            psum2 = ctx.enter_context(tc.tile_pool(name="psum2", bufs=3, space="PSUM"))
            psum1 = ctx.enter_context(tc.tile_pool(name="psum1", bufs=1, space="PSUM"))
            trc = ctx.enter_context(tc.tile_pool(name="trace", bufs=1))
            fin = ctx.enter_context(tc.tile_pool(name="fin", bufs=1))

            sel4_t = consts.tile([128, 128], BF16)
            nc.sync.dma_start(sel4_t[:], sel4_d[:])
            self8_t = consts.tile([96, 8], F32)
            nc.sync.dma_start(self8_t[:], self8_d[:])
            wmat_t = consts.tile([96, COLS], F32)
            nc.sync.dma_start(wmat_t[:], wmat_d[:])

            # trace accumulator [96, CH*G] f32, filled per chunk
            T_t = trc.tile([96, CH * G], F32)

            # expanded block-diag x stationaries, manually double buffered;
            # off-diagonal zeros are written once and persist
            xexp_ts = [
                xexp_p.tile([128, G * 128], BF16, tag=f"xe{i}", name=f"xexp{i}")
                for i in range(2)
            ]
            nc.vector.memset(xexp_ts[0][:], 0.0)
            nc.vector.memset(xexp_ts[1][:], 0.0)

            for ch in range(CH):
                xexp_t = xexp_ts[ch % 2]
                xs = x_d[CS * ch : CS * (ch + 1), :]
                zs = z_d[CS * ch : CS * (ch + 1), :]

                # load x expanded: slot s -> partitions 16s..16s+16, free 128c+16s+j
                xr = xs.rearrange("(c s) (i j) -> s i c j", s=8, i=16)
                xer = xexp_t.rearrange("p (c w) -> p c w", w=128)
                for s in range(8):
                    nc.sync.dma_start(
                        xer[16 * s : 16 * s + 16, :, 16 * s : 16 * s + 16], xr[s]
                    )

                # compact tiles [128, 16G]: partition (s,i), free (c,j)
                xc_t = data.tile([128, 16 * G], BF16, tag="xc")
                nc.sync.dma_start(
                    xc_t.rearrange("p (c j) -> p c j", j=16),
                    xs.rearrange("(c s) (i j) -> (s i) c j", s=8, i=16),
                )
                zc_t = data.tile([128, 16 * G], BF16, tag="zc")
                nc.sync.dma_start(
                    zc_t.rearrange("p (c j) -> p c j", j=16),
                    zs.rearrange("(c s) (i j) -> (s i) c j", s=8, i=16),
                )

                # per-chunk trace psum [96, 16G] f32; strips accumulate 4 powers
                ps_tr = psum1.tile([96, 16 * G], F32, tag="pstr")

                zprev = zc_t  # z^1
                for a in range(1, NSTEP + 1):
                    u = a - 1
                    strip, q = u // 4, u % 4
                    # trace pair product P = z^a (.) x
                    P_t = pows.tile([128, 16 * G], BF16, tag="pair")
                    nc.vector.tensor_tensor(
                        P_t[:], zprev[:], xc_t[:], mybir.AluOpType.mult
                    )
                    # selector fold over i into partition strip rows 8q+s
                    last_q = 3 if strip < 2 else 1
                    for h in range(0, 16 * G, 512):
                        w = min(512, 16 * G - h)
                        nc.tensor.matmul(
                            ps_tr[32 * strip : 32 * strip + 32, h : h + w],
                            sel4_t[:, 32 * q : 32 * q + 32],
                            P_t[:, h : h + w],
                            start=(q == 0),
                            stop=(q == last_q),
                            tile_position=(0, 32 * strip),
                        )

                    if a < NSTEP:
                        # chain step: z^(a+1) = z . z^a per sample, split in
                        # half-chains so TE computes one half while the other
                        # half's PSUM->SBUF copy runs (ACT and DVE in parallel)
                        znext = pows.tile([128, 16 * G], BF16, tag=f"zn{a % 2}")
                        H = G // 2
                        for half in range(2):
                            ps_z = psum2.tile(
                                [128, 16 * H], F32, tag=f"psz{half}",
                                name=f"ps_z{half}",
                            )
                            c0 = half * H
                            for c in range(c0, c0 + H):
                                nc.tensor.matmul(
                                    ps_z[:, 16 * (c - c0) : 16 * (c - c0) + 16],
                                    xexp_t[:, 128 * c : 128 * c + 128],
                                    zprev[:, 16 * c : 16 * c + 16],
                                    start=True,
                                    stop=True,
                                )
                            dst = znext[:, 16 * c0 : 16 * (c0 + H)]
                            if half == 0:
                                nc.scalar.copy(dst, ps_z[:])
                            else:
                                nc.vector.tensor_copy(dst, ps_z[:])
                        zprev = znext

                # fold 16-col segments: [96, (c,16)] -> [96, G]
                nc.vector.tensor_reduce(
                    T_t[:, G * ch : G * (ch + 1)],
                    ps_tr.rearrange("p (c j) -> p c j", j=16),
                    mybir.AxisListType.X,
                    mybir.AluOpType.add,
                )

            # features: S = T/256, G_acc = sum_j W[:,j] * S^(j+1)
            S_t = fin.tile([96, CH * G], F32, tag="S")
            nc.vector.tensor_scalar_mul(S_t[:], T_t[:], 1.0 / 256.0)
            S2_t = fin.tile([96, CH * G], F32, tag="S2")
            nc.vector.tensor_tensor(S2_t[:], S_t[:], S_t[:], mybir.AluOpType.mult)
            S3_t = fin.tile([96, CH * G], F32, tag="S3")
            nc.vector.tensor_tensor(S3_t[:], S2_t[:], S_t[:], mybir.AluOpType.mult)
            S4_t = fin.tile([96, CH * G], F32, tag="S4")
            nc.vector.tensor_tensor(S4_t[:], S2_t[:], S2_t[:], mybir.AluOpType.mult)

            G1_t = fin.tile([96, CH * G], F32, tag="G1")
            nc.vector.tensor_scalar(
                G1_t[:], S_t[:], wmat_t[:, 0:1], None, mybir.AluOpType.mult
            )
            G2_t = fin.tile([96, CH * G], F32, tag="G2")
            nc.vector.scalar_tensor_tensor(
                G2_t[:], S2_t[:], wmat_t[:, 1:2], G1_t[:],
                mybir.AluOpType.mult, mybir.AluOpType.add,
            )
            G3_t = fin.tile([96, CH * G], F32, tag="G3")
            nc.vector.scalar_tensor_tensor(
                G3_t[:], S3_t[:], wmat_t[:, 2:3], G2_t[:],
                mybir.AluOpType.mult, mybir.AluOpType.add,
            )
            G4_t = fin.tile([96, CH * G], F32, tag="G4")
            nc.vector.scalar_tensor_tensor(
                G4_t[:], S4_t[:], wmat_t[:, 3:4], G3_t[:],
                mybir.AluOpType.mult, mybir.AluOpType.add,
            )

            # fold the 10 power-rows into 8 sample rows: out[s, (ch,c)]
            ps_out = psum1.tile([8, CH * G], F32, tag="pstr", name="ps_out")
            for h in range(0, CH * G, 512):
                w = min(512, CH * G - h)
                nc.tensor.matmul(
                    ps_out[:, h : h + w],
                    self8_t[:],
                    G4_t[:, h : h + w],
                    start=True,
                    stop=True,
                )
            out_sb = fin.tile([8, CH * G], F32, tag="outsb")
            nc.vector.tensor_copy(out_sb[:], ps_out[:])
            # out[b], b = 512*ch + 8*c + s; psum cols are (ch,c)
            nc.sync.dma_start(
                out_d.rearrange("(ch c s) -> s (ch c)", ch=CH, c=G, s=8),
                out_sb[:],
            )

    nc.compile()
    return nc


def _prep_inputs(x: np.ndarray, coef: np.ndarray):
    x = np.ascontiguousarray(x, dtype=np.float32).reshape(B, N, N)
    xb = x.astype(ml_dtypes.bfloat16)
    zb = np.ascontiguousarray(xb.transpose(0, 2, 1))
    xb = xb.reshape(M, BS, 256)
    zb = zb.reshape(M, BS, 256)
    sel4, self8, wmat = _consts(np.asarray(coef, dtype=np.float32))
    return [
        {"x": xb[i], "z": zb[i], "sel4": sel4, "self8": self8, "wmat": wmat}
        for i in range(M)
    ]


def kernel(x: np.ndarray, coef: np.ndarray) -> np.ndarray:
    global _cached
    if _cached is None:
        _cached = build()
    in_maps = _prep_inputs(x, coef)
    res = run_bass_kernel_spmd(_cached, in_maps, core_ids=list(range(M)))
    out = np.concatenate(
        [np.asarray(res.results[i]["out"]).reshape(BS) for i in range(M)]
    )
    return out.astype(np.float32)


if __name__ == "__main__":
    rng = np.random.default_rng(0)
    x = (rng.standard_normal((B, N, N)) * 0.5).astype(np.float32)
    coef = (rng.standard_normal((ROWS, COLS)) * np.sqrt(0.5)).astype(np.float32)
    got = kernel(x, coef)
    print(got[:8])


# revision 10
# speedup vs baseline: 1.1015x; 1.1001x over previous
"""Distributed trace-polynomial Bass kernel for trn2 (8 NeuronCores).

Problem: x [65536,16,16] f32, coef [10,4].
  t_u(b) = trace(x_b^(u+2)), u=0..9
  out[b] = sum_uj coef[u,j] * t_u^(j+1) / 256^(u+j+1)

Device math (per core, 8192 samples, all sample data bf16):
  z := x^T (per-sample transpose, prepared host-side, shipped as input)
  chain: z^(a+1) = z . z^a   (TE matmul, stationary = x expanded to an
         8-sample block-diagonal [128,128]; out = lhsT^T @ rhs = z . rhs)
  traces: tr(x^(a+1)) = <z^a, x>_F per sample
         = elementwise product (DVE) -> partition fold over i (TE with a
           constant selector stationary) -> 16-col segment fold (DVE
           tensor_reduce axis=X).
  features: S = t/256; out = sum_uj coef[u,j] 256^-u S^(j+1)  (DVE + TE fold)

Sharding: pure data parallel, batch split 8 ways across cores; coef only
enters via a tiny host-computed weight matrix; output gathered to [65536].
bf16 storage keeps rel-l2 ~2e-3 vs f64 reference (tolerance 2e-2).
"""

import numpy as np
from contextlib import ExitStack

import ml_dtypes

from concourse import bass, bacc, tile, mybir
from concourse.bass_utils import run_bass_kernel_spmd

B, N = 65536, 16
ROWS, COLS = 10, 4
M = 8           # cores
BS = B // M     # 8192 samples per core
CH = 16         # chunks per core
CS = BS // CH   # 512 samples per chunk
G = CS // 8     # 64 groups (of 8 samples) per chunk
NSTEP = 10      # pairs a=1..10 -> traces t_2..t_11 (9 chain matmuls)

BF16 = mybir.dt.bfloat16
F32 = mybir.dt.float32

_cached = None


def _consts(coef: np.ndarray):
    # row layout of the 96-partition trace tile: r = 32*(u//4) + 8*(u%4) + s
    wmat = np.zeros((96, COLS), np.float32)
    self8 = np.zeros((96, 8), np.float32)
    for u in range(ROWS):
        base = 32 * (u // 4) + 8 * (u % 4)
        for s in range(8):
            wmat[base + s, :] = coef[u, :] * (256.0 ** (-u))
            self8[base + s, s] = 1.0
    # sel4[:, 32q:32q+32]: lhsT mapping partition (s,i) -> within-slice col 8q+s
    sel4 = np.zeros((128, 128), np.float32)
    for q in range(4):
        for s in range(8):
            for i in range(16):
                sel4[16 * s + i, 32 * q + 8 * q + s] = 1.0
    return (
        sel4.astype(ml_dtypes.bfloat16),
        self8.astype(np.float32),
        wmat,
    )


def build():
    nc = bacc.Bacc("TRN2", target_bir_lowering=False, debug=False, num_devices=M)

    x_d = nc.dram_tensor("x", [BS, 256], BF16, kind="ExternalInput").ap()
    z_d = nc.dram_tensor("z", [BS, 256], BF16, kind="ExternalInput").ap()
    sel4_d = nc.dram_tensor("sel4", [128, 128], BF16, kind="ExternalInput").ap()
    self8_d = nc.dram_tensor("self8", [96, 8], F32, kind="ExternalInput").ap()
    wmat_d = nc.dram_tensor("wmat", [96, COLS], F32, kind="ExternalInput").ap()
    out_d = nc.dram_tensor("out", [BS], F32, kind="ExternalOutput").ap()

    with tile.TileContext(nc) as tc:
        with ExitStack() as ctx:
            consts = ctx.enter_context(tc.tile_pool(name="consts", bufs=1))
            xexp_p = ctx.enter_context(tc.tile_pool(name="xexp", bufs=1))
            data = ctx.enter_context(tc.tile_pool(name="data", bufs=3))
            pows = ctx.enter_context(tc.tile_pool(name="pows", bufs=3))
            psum2 = ctx.enter_context(tc.tile_pool(name="psum2", bufs=3, space="PSUM"))
            psum1 = ctx.enter_context(tc.tile_pool(name="psum1", bufs=1, space="PSUM"))
            trc = ctx.enter_context(tc.tile_pool(name="trace", bufs=1))
            fin = ctx.enter_context(tc.tile_pool(name="fin", bufs=1))

            sel4_t = consts.tile([128, 128], BF16)
            nc.sync.dma_start(sel4_t[:], sel4_d[:])
            self8_t = consts.tile([96, 8], F32)
            nc.sync.dma_start(self8_t[:], self8_d[:])
            wmat_t = consts.tile([96, COLS], F32)
            nc.sync.dma_start(wmat_t[:], wmat_d[:])

            # trace accumulator [96, CH*G] f32, filled per chunk
            T_t = trc.tile([96, CH * G], F32)

            # expanded block-diag x stationaries, manually double buffered;
            # off-diagonal zeros are written once and persist
            xexp_ts = [
                xexp_p.tile([128, G * 128], BF16, tag=f"xe{i}", name=f"xexp{i}")
                for i in range(2)
            ]
            nc.vector.memset(xexp_ts[0][:], 0.0)
            nc.vector.memset(xexp_ts[1][:], 0.0)

            for ch in range(CH):
                xexp_t = xexp_ts[ch % 2]
                xs = x_d[CS * ch : CS * (ch + 1), :]
                zs = z_d[CS * ch : CS * (ch + 1), :]

                # load x expanded: slot s -> partitions 16s..16s+16, free 128c+16s+j
                xr = xs.rearrange("(c s) (i j) -> s i c j", s=8, i=16)
                xer = xexp_t.rearrange("p (c w) -> p c w", w=128)
                for s in range(8):
                    nc.sync.dma_start(
                        xer[16 * s : 16 * s + 16, :, 16 * s : 16 * s + 16], xr[s]
                    )

                # compact tiles [128, 16G]: partition (s,i), free (c,j)
                xc_t = data.tile([128, 16 * G], BF16, tag="xc")
                nc.sync.dma_start(
                    xc_t.rearrange("p (c j) -> p c j", j=16),
                    xs.rearrange("(c s) (i j) -> (s i) c j", s=8, i=16),
                )
                zc_t = data.tile([128, 16 * G], BF16, tag="zc")
                nc.sync.dma_start(
                    zc_t.rearrange("p (c j) -> p c j", j=16),
                    zs.rearrange("(c s) (i j) -> (s i) c j", s=8, i=16),
                )

                # per-chunk trace psum [96, 16G] f32; strips accumulate 4 powers
                ps_tr = psum1.tile([96, 16 * G], F32, tag="pstr")

                zprev = zc_t  # z^1
                for a in range(1, NSTEP + 1):
                    u = a - 1
                    strip, q = u // 4, u % 4
                    # trace pair product P = z^a (.) x
                    P_t = pows.tile([128, 16 * G], BF16, tag="pair")
                    nc.vector.tensor_tensor(
                        P_t[:], zprev[:], xc_t[:], mybir.AluOpType.mult
                    )
                    # selector fold over i into partition strip rows 8q+s
                    last_q = 3 if strip < 2 else 1
                    for h in range(0, 16 * G, 512):
                        w = min(512, 16 * G - h)
                        nc.tensor.matmul(
                            ps_tr[32 * strip : 32 * strip + 32, h : h + w],
                            sel4_t[:, 32 * q : 32 * q + 32],
                            P_t[:, h : h + w],
                            start=(q == 0),
                            stop=(q == last_q),
                            tile_position=(0, 32 * strip),
                        )

                    if a < NSTEP:
                        # chain step: z^(a+1) = z . z^a per sample
                        ps_z = psum2.tile([128, 16 * G], F32, tag="psz")
                        for c in range(G):
                            nc.tensor.matmul(
                                ps_z[:, 16 * c : 16 * c + 16],
                                xexp_t[:, 128 * c : 128 * c + 128],
                                zprev[:, 16 * c : 16 * c + 16],
                                start=True,
                                stop=True,
                            )
                        # copy halves on ACT and DVE in parallel to halve the
                        # chain-critical-path latency of the PSUM->SBUF hop
                        znext = pows.tile([128, 16 * G], BF16, tag=f"zn{a % 2}")
                        HW = 8 * G
                        nc.scalar.copy(znext[:, 0:HW], ps_z[:, 0:HW])
                        nc.vector.tensor_copy(znext[:, HW:], ps_z[:, HW:])
                        zprev = znext

                # fold 16-col segments: [96, (c,16)] -> [96, G]
                nc.vector.tensor_reduce(
                    T_t[:, G * ch : G * (ch + 1)],
                    ps_tr.rearrange("p (c j) -> p c j", j=16),
                    mybir.AxisListType.X,
                    mybir.AluOpType.add,
                )

            # features: S = T/256, G_acc = sum_j W[:,j] * S^(j+1)
            S_t = fin.tile([96, CH * G], F32, tag="S")
            nc.vector.tensor_scalar_mul(S_t[:], T_t[:], 1.0 / 256.0)
            S2_t = fin.tile([96, CH * G], F32, tag="S2")
            nc.vector.tensor_tensor(S2_t[:], S_t[:], S_t[:], mybir.AluOpType.mult)
            S3_t = fin.tile([96, CH * G], F32, tag="S3")
            nc.vector.tensor_tensor(S3_t[:], S2_t[:], S_t[:], mybir.AluOpType.mult)
            S4_t = fin.tile([96, CH * G], F32, tag="S4")
            nc.vector.tensor_tensor(S4_t[:], S2_t[:], S2_t[:], mybir.AluOpType.mult)

            G1_t = fin.tile([96, CH * G], F32, tag="G1")
            nc.vector.tensor_scalar(
                G1_t[:], S_t[:], wmat_t[:, 0:1], None, mybir.AluOpType.mult
            )
            G2_t = fin.tile([96, CH * G], F32, tag="G2")
            nc.vector.scalar_tensor_tensor(
                G2_t[:], S2_t[:], wmat_t[:, 1:2], G1_t[:],
                mybir.AluOpType.mult, mybir.AluOpType.add,
            )
            G3_t = fin.tile([96, CH * G], F32, tag="G3")
            nc.vector.scalar_tensor_tensor(
                G3_t[:], S3_t[:], wmat_t[:, 2:3], G2_t[:],
                mybir.AluOpType.mult, mybir.AluOpType.add,
            )
            G4_t = fin.tile([96, CH * G], F32, tag="G4")
            nc.vector.scalar_tensor_tensor(
                G4_t[:], S4_t[:], wmat_t[:, 3:4], G3_t[:],
                mybir.AluOpType.mult, mybir.AluOpType.add,
            )

            # fold the 10 power-rows into 8 sample rows: out[s, (ch,c)]
            ps_out = psum1.tile([8, CH * G], F32, tag="pstr", name="ps_out")
            for h in range(0, CH * G, 512):
                w = min(512, CH * G - h)
                nc.tensor.matmul(
                    ps_out[:, h : h + w],
                    self8_t[:],
                    G4_t[:, h : h + w],
                    start=True,
                    stop=True,
                )
            out_sb = fin.tile([8, CH * G], F32, tag="outsb")
            nc.vector.tensor_copy(out_sb[:], ps_out[:])
            # out[b], b = 512*ch + 8*c + s; psum cols are (ch,c)
            nc.sync.dma_start(
                out_d.rearrange("(ch c s) -> s (ch c)", ch=CH, c=G, s=8),
                out_sb[:],
            )

    nc.compile()
    return nc


def _prep_inputs(x: np.ndarray, coef: np.ndarray):
    x = np.ascontiguousarray(x, dtype=np.float32).reshape(B, N, N)
    xb = x.astype(ml_dtypes.bfloat16)
    zb = np.ascontiguousarray(xb.transpose(0, 2, 1))
    xb = xb.reshape(M, BS, 256)
    zb = zb.reshape(M, BS, 256)
    sel4, self8, wmat = _consts(np.asarray(coef, dtype=np.float32))
    return [
        {"x": xb[i], "z": zb[i], "sel4": sel4, "self8": self8, "wmat": wmat}
        for i in range(M)
    ]


def kernel(x: np.ndarray, coef: np.ndarray) -> np.ndarray:
    global _cached
    if _cached is None:
        _cached = build()
    in_maps = _prep_inputs(x, coef)
    res = run_bass_kernel_spmd(_cached, in_maps, core_ids=list(range(M)))
    out = np.concatenate(
        [np.asarray(res.results[i]["out"]).reshape(BS) for i in range(M)]
    )
    return out.astype(np.float32)


if __name__ == "__main__":
    rng = np.random.default_rng(0)
    x = (rng.standard_normal((B, N, N)) * 0.5).astype(np.float32)
    coef = (rng.standard_normal((ROWS, COLS)) * np.sqrt(0.5)).astype(np.float32)
    got = kernel(x, coef)
    print(got[:8])


# revision 11
# speedup vs baseline: 1.1243x; 1.0207x over previous
"""Distributed trace-polynomial Bass kernel for trn2 (8 NeuronCores).

Problem: x [65536,16,16] f32, coef [10,4].
  t_u(b) = trace(x_b^(u+2)), u=0..9
  out[b] = sum_uj coef[u,j] * t_u^(j+1) / 256^(u+j+1)

Device math (per core, 8192 samples, all sample data bf16):
  z := x^T (per-sample transpose, prepared host-side, shipped as input)
  chain: z^(a+1) = z . z^a   (TE matmul, stationary = x expanded to an
         8-sample block-diagonal [128,128]; out = lhsT^T @ rhs = z . rhs)
  traces: tr(x^(a+1)) = <z^a, x>_F per sample
         = elementwise product (DVE) -> partition fold over i (TE with a
           constant selector stationary) -> 16-col segment fold (DVE
           tensor_reduce axis=X).
  features: S = t/256; out = sum_uj coef[u,j] 256^-u S^(j+1)  (DVE + TE fold)

Sharding: pure data parallel, batch split 8 ways across cores; coef only
enters via a tiny host-computed weight matrix; output gathered to [65536].
bf16 storage keeps rel-l2 ~2e-3 vs f64 reference (tolerance 2e-2).
"""

import numpy as np
from contextlib import ExitStack

import ml_dtypes

from concourse import bass, bacc, tile, mybir
from concourse.bass_utils import run_bass_kernel_spmd

B, N = 65536, 16
ROWS, COLS = 10, 4
M = 8           # cores
BS = B // M     # 8192 samples per core
CH = 16         # chunks per core
CS = BS // CH   # 512 samples per chunk
G = CS // 8     # 64 groups (of 8 samples) per chunk
NSTEP = 10      # pairs a=1..10 -> traces t_2..t_11 (9 chain matmuls)

BF16 = mybir.dt.bfloat16
F32 = mybir.dt.float32

_cached = None


def _consts(coef: np.ndarray):
    # row layout of the 96-partition trace tile: r = 32*(u//4) + 8*(u%4) + s
    wmat = np.zeros((96, COLS), np.float32)
    self8 = np.zeros((96, 8), np.float32)
    for u in range(ROWS):
        base = 32 * (u // 4) + 8 * (u % 4)
        for s in range(8):
            wmat[base + s, :] = coef[u, :] * (256.0 ** (-u))
            self8[base + s, s] = 1.0
    # sel4[:, 32q:32q+32]: lhsT mapping partition (s,i) -> within-slice col 8q+s
    sel4 = np.zeros((128, 128), np.float32)
    for q in range(4):
        for s in range(8):
            for i in range(16):
                sel4[16 * s + i, 32 * q + 8 * q + s] = 1.0
    return (
        sel4.astype(ml_dtypes.bfloat16),
        self8.astype(np.float32),
        wmat,
    )


def build():
    nc = bacc.Bacc("TRN2", target_bir_lowering=False, debug=False, num_devices=M)

    x_d = nc.dram_tensor("x", [BS, 256], BF16, kind="ExternalInput").ap()
    z_d = nc.dram_tensor("z", [BS, 256], BF16, kind="ExternalInput").ap()
    sel4_d = nc.dram_tensor("sel4", [128, 128], BF16, kind="ExternalInput").ap()
    self8_d = nc.dram_tensor("self8", [96, 8], F32, kind="ExternalInput").ap()
    wmat_d = nc.dram_tensor("wmat", [96, COLS], F32, kind="ExternalInput").ap()
    out_d = nc.dram_tensor("out", [BS], F32, kind="ExternalOutput").ap()

    with tile.TileContext(nc) as tc:
        with ExitStack() as ctx:
            consts = ctx.enter_context(tc.tile_pool(name="consts", bufs=1))
            xexp_p = ctx.enter_context(tc.tile_pool(name="xexp", bufs=1))
            data = ctx.enter_context(tc.tile_pool(name="data", bufs=3))
            pows = ctx.enter_context(tc.tile_pool(name="pows", bufs=3))
            psum2 = ctx.enter_context(tc.tile_pool(name="psum2", bufs=3, space="PSUM"))
            psum1 = ctx.enter_context(tc.tile_pool(name="psum1", bufs=1, space="PSUM"))
            trc = ctx.enter_context(tc.tile_pool(name="trace", bufs=1))
            fin = ctx.enter_context(tc.tile_pool(name="fin", bufs=1))

            sel4_t = consts.tile([128, 128], BF16)
            nc.sync.dma_start(sel4_t[:], sel4_d[:])
            self8_t = consts.tile([96, 8], F32)
            nc.sync.dma_start(self8_t[:], self8_d[:])
            wmat_t = consts.tile([96, COLS], F32)
            nc.sync.dma_start(wmat_t[:], wmat_d[:])

            # trace accumulator [96, CH*G] f32, filled per chunk
            T_t = trc.tile([96, CH * G], F32)

            # expanded block-diag x stationaries, manually double buffered;
            # off-diagonal zeros are written once and persist
            xexp_ts = [
                xexp_p.tile([128, G * 128], BF16, tag=f"xe{i}", name=f"xexp{i}")
                for i in range(2)
            ]
            nc.gpsimd.memset(xexp_ts[0][:], 0.0)
            nc.gpsimd.memset(xexp_ts[1][:], 0.0)

            for ch in range(CH):
                xexp_t = xexp_ts[ch % 2]
                xs = x_d[CS * ch : CS * (ch + 1), :]
                zs = z_d[CS * ch : CS * (ch + 1), :]

                # load x expanded: slot s -> partitions 16s..16s+16, free 128c+16s+j
                xr = xs.rearrange("(c s) (i j) -> s i c j", s=8, i=16)
                xer = xexp_t.rearrange("p (c w) -> p c w", w=128)
                for s in range(8):
                    nc.sync.dma_start(
                        xer[16 * s : 16 * s + 16, :, 16 * s : 16 * s + 16], xr[s]
                    )

                # compact tiles [128, 16G]: partition (s,i), free (c,j)
                xc_t = data.tile([128, 16 * G], BF16, tag="xc")
                nc.sync.dma_start(
                    xc_t.rearrange("p (c j) -> p c j", j=16),
                    xs.rearrange("(c s) (i j) -> (s i) c j", s=8, i=16),
                )
                zc_t = data.tile([128, 16 * G], BF16, tag="zc")
                nc.sync.dma_start(
                    zc_t.rearrange("p (c j) -> p c j", j=16),
                    zs.rearrange("(c s) (i j) -> (s i) c j", s=8, i=16),
                )

                # per-chunk trace psum [96, 16G] f32; strips accumulate 4 powers
                ps_tr = psum1.tile([96, 16 * G], F32, tag="pstr")

                zprev = zc_t  # z^1
                for a in range(1, NSTEP + 1):
                    u = a - 1
                    strip, q = u // 4, u % 4
                    # trace pair product P = z^a (.) x
                    P_t = pows.tile([128, 16 * G], BF16, tag="pair")
                    nc.vector.tensor_tensor(
                        P_t[:], zprev[:], xc_t[:], mybir.AluOpType.mult
                    )
                    # selector fold over i into partition strip rows 8q+s
                    last_q = 3 if strip < 2 else 1
                    for h in range(0, 16 * G, 512):
                        w = min(512, 16 * G - h)
                        nc.tensor.matmul(
                            ps_tr[32 * strip : 32 * strip + 32, h : h + w],
                            sel4_t[:, 32 * q : 32 * q + 32],
                            P_t[:, h : h + w],
                            start=(q == 0),
                            stop=(q == last_q),
                            tile_position=(0, 32 * strip),
                        )

                    if a < NSTEP:
                        # chain step: z^(a+1) = z . z^a per sample
                        ps_z = psum2.tile([128, 16 * G], F32, tag="psz")
                        for c in range(G):
                            nc.tensor.matmul(
                                ps_z[:, 16 * c : 16 * c + 16],
                                xexp_t[:, 128 * c : 128 * c + 128],
                                zprev[:, 16 * c : 16 * c + 16],
                                start=True,
                                stop=True,
                            )
                        # copy halves on ACT and DVE in parallel to halve the
                        # chain-critical-path latency of the PSUM->SBUF hop
                        znext = pows.tile([128, 16 * G], BF16, tag=f"zn{a % 2}")
                        HW = 8 * G
                        nc.scalar.copy(znext[:, 0:HW], ps_z[:, 0:HW])
                        nc.vector.tensor_copy(znext[:, HW:], ps_z[:, HW:])
                        zprev = znext

                # fold 16-col segments: [96, (c,16)] -> [96, G]
                nc.vector.tensor_reduce(
                    T_t[:, G * ch : G * (ch + 1)],
                    ps_tr.rearrange("p (c j) -> p c j", j=16),
                    mybir.AxisListType.X,
                    mybir.AluOpType.add,
                )

            # features: S = T/256, G_acc = sum_j W[:,j] * S^(j+1)
            S_t = fin.tile([96, CH * G], F32, tag="S")
            nc.vector.tensor_scalar_mul(S_t[:], T_t[:], 1.0 / 256.0)
            S2_t = fin.tile([96, CH * G], F32, tag="S2")
            nc.vector.tensor_tensor(S2_t[:], S_t[:], S_t[:], mybir.AluOpType.mult)
            S3_t = fin.tile([96, CH * G], F32, tag="S3")
            nc.vector.tensor_tensor(S3_t[:], S2_t[:], S_t[:], mybir.AluOpType.mult)
            S4_t = fin.tile([96, CH * G], F32, tag="S4")
            nc.vector.tensor_tensor(S4_t[:], S2_t[:], S2_t[:], mybir.AluOpType.mult)

            G1_t = fin.tile([96, CH * G], F32, tag="G1")
            nc.vector.tensor_scalar(
                G1_t[:], S_t[:], wmat_t[:, 0:1], None, mybir.AluOpType.mult
            )
            G2_t = fin.tile([96, CH * G], F32, tag="G2")
            nc.vector.scalar_tensor_tensor(
                G2_t[:], S2_t[:], wmat_t[:, 1:2], G1_t[:],
                mybir.AluOpType.mult, mybir.AluOpType.add,
            )
            G3_t = fin.tile([96, CH * G], F32, tag="G3")
            nc.vector.scalar_tensor_tensor(
                G3_t[:], S3_t[:], wmat_t[:, 2:3], G2_t[:],
                mybir.AluOpType.mult, mybir.AluOpType.add,
            )
            G4_t = fin.tile([96, CH * G], F32, tag="G4")
            nc.vector.scalar_tensor_tensor(
                G4_t[:], S4_t[:], wmat_t[:, 3:4], G3_t[:],
                mybir.AluOpType.mult, mybir.AluOpType.add,
            )

            # fold the 10 power-rows into 8 sample rows: out[s, (ch,c)]
            ps_out = psum1.tile([8, CH * G], F32, tag="pstr", name="ps_out")
            for h in range(0, CH * G, 512):
                w = min(512, CH * G - h)
                nc.tensor.matmul(
                    ps_out[:, h : h + w],
                    self8_t[:],
                    G4_t[:, h : h + w],
                    start=True,
                    stop=True,
                )
            out_sb = fin.tile([8, CH * G], F32, tag="outsb")
            nc.vector.tensor_copy(out_sb[:], ps_out[:])
            # out[b], b = 512*ch + 8*c + s; psum cols are (ch,c)
            nc.sync.dma_start(
                out_d.rearrange("(ch c s) -> s (ch c)", ch=CH, c=G, s=8),
                out_sb[:],
            )

    nc.compile()
    return nc


def _prep_inputs(x: np.ndarray, coef: np.ndarray):
    x = np.ascontiguousarray(x, dtype=np.float32).reshape(B, N, N)
    xb = x.astype(ml_dtypes.bfloat16)
    zb = np.ascontiguousarray(xb.transpose(0, 2, 1))
    xb = xb.reshape(M, BS, 256)
    zb = zb.reshape(M, BS, 256)
    sel4, self8, wmat = _consts(np.asarray(coef, dtype=np.float32))
    return [
        {"x": xb[i], "z": zb[i], "sel4": sel4, "self8": self8, "wmat": wmat}
        for i in range(M)
    ]


def kernel(x: np.ndarray, coef: np.ndarray) -> np.ndarray:
    global _cached
    if _cached is None:
        _cached = build()
    in_maps = _prep_inputs(x, coef)
    res = run_bass_kernel_spmd(_cached, in_maps, core_ids=list(range(M)))
    out = np.concatenate(
        [np.asarray(res.results[i]["out"]).reshape(BS) for i in range(M)]
    )
    return out.astype(np.float32)


if __name__ == "__main__":
    rng = np.random.default_rng(0)
    x = (rng.standard_normal((B, N, N)) * 0.5).astype(np.float32)
    coef = (rng.standard_normal((ROWS, COLS)) * np.sqrt(0.5)).astype(np.float32)
    got = kernel(x, coef)
    print(got[:8])
